# revision 2
# baseline (speedup 1.0000x reference)
"""Per-batch covariance + triu gather on 8 Trainium2 NeuronCores.

Problem: inputs [64, 4096, 256] f32 -> out [64, 32896] f32 where
out[b] = triu(cov(inputs[b])) in row-major order and
cov = (xc^T @ xc) / N with xc = x - mean(x, axis=0).

Strategy (data-parallel, 8 batches per core):
- y = x / 64 (exact scaling, 64 = sqrt(N)) is produced by the DVE pass that
  also rounds f32 -> f32r (required for single-pass-rate PE matmuls).
- An appended column of 1/64 makes the Gram accumulation produce the mean
  vector in the same matmuls: psum[d, 256] = sum_n y[n,d]/64 = mu[d], while
  psum[d, e] = sum_n y[n,d] y[n,e] = G[d,e]/N.
- mean correction: cov = G/N - mu mu^T is applied with one rank-1 matmul per
  128-row half accumulating into the same PSUM (lhsT = -mu, rhs = mu, K=1).
- triu extraction: either 256 row-tail DMAs (one per matrix row, covering all
  8 batches each) or a full-cov DMA with the gather done by the caller.
"""

import os
import numpy as np

B, N, D = 64, 4096, 256
NCORES = 8
BPC = B // NCORES          # batches per core
TRI = D * (D + 1) // 2     # 32896
CHUNKS = N // 128          # 32
SCALE = 1.0 / 64.0         # 1/sqrt(N)

TRIU_MODE = os.environ.get("COV_TRIU_MODE", "rowdma")  # "rowdma" | "host"

_cache = {}


def _build(triu_mode):
    import concourse.bacc as bacc
    import concourse.mybir as mybir
    from concourse.tile import TileContext

    F32 = mybir.dt.float32
    F32R = mybir.dt.float32r

    nc = bacc.Bacc("TRN2", target_bir_lowering=False)
    x = nc.dram_tensor("x", [BPC, N, D], F32, kind="ExternalInput")
    ident = nc.dram_tensor("ident", [128, 128], F32, kind="ExternalInput")
    if triu_mode == "host":
        out = nc.dram_tensor("out", [BPC, D, D], F32, kind="ExternalOutput")
    else:
        out = nc.dram_tensor("out", [BPC, TRI], F32, kind="ExternalOutput")

    # x[b] rows are (chunk, partition); one DMA per half-batch of 16 chunks
    xv = x.rearrange("b (h c p) d -> b h p c d", h=2, p=128)

    with TileContext(nc) as tc:
        with (
            tc.tile_pool(name="cst", bufs=1) as cst,
            tc.tile_pool(name="xin", bufs=6) as xinp,
            tc.tile_pool(name="sb", bufs=2) as sb,
            tc.tile_pool(name="ps", bufs=2, space="PSUM") as ps,
        ):
            ident_sb = cst.tile([128, 128], F32)
            nc.sync.dma_start(ident_sb, ident[:, :])
            ones2 = cst.tile([128, 2], F32)
            nc.vector.memset(ones2, 1.0)

            rnds = []
            for j in range(8):
                t = cst.tile([128, 258], F32R, name=f"rnd{j}")
                nc.vector.tensor_scalar_mul(t[:, 256:258], ones2, SCALE)
                rnds.append(t)

            # packed covariance halves for all 8 batches: [p, (b e)]
            covA = cst.tile([128, BPC * 256], F32)
            covB = cst.tile([128, BPC * 256], F32)

            pstate = {}

            def emit_chunks(b):
                ps0 = ps.tile([128, 258], F32, name=f"ps0_{b}", tag="ps0")
                ps1 = ps.tile([128, 258], F32, name=f"ps1_{b}", tag="ps1")
                halves = []
                for h in range(2):
                    xt = xinp.tile([128, 16 * 256], F32, name=f"xt{b}_{h}",
                                   tag="xt")
                    nc.sync.dma_start(
                        xt.rearrange("p (c d) -> p c d", d=256), xv[b, h])
                    halves.append(xt)
                for c in range(CHUNKS):
                    xt = halves[c // 16]
                    sl = xt[:, (c % 16) * 256:(c % 16 + 1) * 256]
                    t = rnds[(b * CHUNKS + c) % len(rnds)]
                    nc.vector.tensor_scalar_mul(t[:, 0:256], sl, SCALE)
                    nc.tensor.matmul(ps0, t[:, 0:128], t, start=(c == 0),
                                     stop=False, skip_group_check=True)
                    nc.tensor.matmul(ps1, t[:, 128:256], t, start=(c == 0),
                                     stop=False, skip_group_check=True)
                pstate[b] = (ps0, ps1)

            def emit_epilogue(b):
                ps0, ps1 = pstate.pop(b)
                scol = sb.tile([128, 2], F32, name=f"scol{b}", tag="scol")
                nc.scalar.copy(scol[:, 0:1], ps0[:, 256:257])
                nc.scalar.copy(scol[:, 1:2], ps1[:, 256:257])
                pst = ps.tile([1, 256], F32, name=f"pst{b}", tag="pst")
                nc.tensor.transpose(pst[0:1, 0:128], scol[:, 0:1], ident_sb)
                nc.tensor.transpose(pst[0:1, 128:256], scol[:, 1:2], ident_sb)
                murow = sb.tile([1, 256], F32R, name=f"mur{b}", tag="mur")
                nmurow = sb.tile([1, 256], F32R, name=f"nmur{b}", tag="nmur")
                nc.scalar.copy(murow, pst[0:1, :])
                nc.scalar.mul(nmurow, pst[0:1, :], -1.0)
                nc.tensor.matmul(ps0[:, 0:256], nmurow[0:1, 0:128],
                                 murow[0:1, :], start=False, stop=True,
                                 skip_group_check=True)
                nc.tensor.matmul(ps1[:, 0:256], nmurow[0:1, 128:256],
                                 murow[0:1, :], start=False, stop=True,
                                 skip_group_check=True)
                nc.vector.tensor_copy(covA[:, b * 256:(b + 1) * 256],
                                      ps0[:, 0:256])
                nc.vector.tensor_copy(covB[:, b * 256:(b + 1) * 256],
                                      ps1[:, 0:256])
                if triu_mode == "host":
                    nc.sync.dma_start(out[b, 0:128, :],
                                      covA[:, b * 256:(b + 1) * 256])
                    nc.scalar.dma_start(out[b, 128:256, :],
                                        covB[:, b * 256:(b + 1) * 256])

            for b in range(BPC):
                emit_chunks(b)
                if b >= 1:
                    emit_epilogue(b - 1)
            emit_epilogue(BPC - 1)

            if triu_mode == "rowdma":
                covA3 = covA.rearrange("p (b e) -> p b e", e=256)
                covB3 = covB.rearrange("p (b e) -> p b e", e=256)
                engines = [nc.sync, nc.scalar]
                for d in range(D):
                    half = covA3 if d < 128 else covB3
                    p = d % 128
                    ln = D - d
                    off = d * D - (d * (d - 1)) // 2
                    src = half[p:p + 1, :, d:D]     # [1, 8, ln], 1 partition
                    dst = out[:, off:off + ln]      # [8, ln]
                    engines[d % 2].dma_start(dst, src)

    nc.finalize()
    return nc


def _get_nc(triu_mode):
    if triu_mode not in _cache:
        _cache[triu_mode] = _build(triu_mode)
    return _cache[triu_mode]


_TRIU_ROWS = None


def _host_gather(cov_full):
    # cov_full: [B, D, D] -> [B, TRI] row-major upper triangle
    global _TRIU_ROWS
    if _TRIU_ROWS is None:
        _TRIU_ROWS = np.triu_indices(D)
    iu, ju = _TRIU_ROWS
    return cov_full[:, iu, ju]


def kernel(**inputs):
    from concourse.bass_utils import run_bass_kernel_spmd

    x = np.asarray(inputs["inputs"], dtype=np.float32)
    assert x.shape == (B, N, D), x.shape
    ident = np.eye(128, dtype=np.float32)
    nc = _get_nc(TRIU_MODE)
    in_maps = [
        {"x": np.ascontiguousarray(x[c * BPC:(c + 1) * BPC]), "ident": ident}
        for c in range(NCORES)
    ]
    res = run_bass_kernel_spmd(nc, in_maps, core_ids=list(range(NCORES)))
    outs = [res.results[c]["out"] for c in range(NCORES)]
    full = np.concatenate(outs, axis=0)
    if TRIU_MODE == "host":
        return _host_gather(full)
    return full.reshape(B, TRI)


# revision 4
# speedup vs baseline: 1.5656x; 1.5656x over previous
"""Per-batch covariance + triu gather on 8 Trainium2 NeuronCores.

Problem: inputs [64, 4096, 256] f32 -> out [64, 32896] f32 where
out[b] = triu(cov(inputs[b])) in row-major order and
cov = (xc^T @ xc) / N with xc = x - mean(x, axis=0).

Strategy (data-parallel, 8 batches per core):
- y = x / 64 (exact scaling, 64 = sqrt(N)) is produced by the DVE pass that
  also rounds f32 -> f32r (required for single-pass-rate PE matmuls).
- An appended column of 1/64 makes the Gram accumulation produce the mean
  vector in the same matmuls: psum[d, 256] = sum_n y[n,d]/64 = mu[d], while
  psum[d, e] = sum_n y[n,d] y[n,e] = G[d,e]/N.
- mean correction: cov = G/N - mu mu^T is applied with one rank-1 matmul per
  128-row half accumulating into the same PSUM (lhsT = -mu, rhs = mu, K=1).
- triu extraction: either 256 row-tail DMAs (one per matrix row, covering all
  8 batches each) or a full-cov DMA with the gather done by the caller.
"""

import os
import numpy as np

B, N, D = 64, 4096, 256
NCORES = 8
BPC = B // NCORES          # batches per core
TRI = D * (D + 1) // 2     # 32896
CHUNKS = N // 128          # 32
SCALE = 1.0 / 64.0         # 1/sqrt(N)

TRIU_MODE = os.environ.get("COV_TRIU_MODE", "rowdma")  # "rowdma" | "host"

_cache = {}


def _build(triu_mode):
    import concourse.bacc as bacc
    import concourse.mybir as mybir
    from concourse.tile import TileContext

    F32 = mybir.dt.float32
    F32R = mybir.dt.float32r

    nc = bacc.Bacc("TRN2", target_bir_lowering=False)
    x = nc.dram_tensor("x", [BPC, N, D], F32, kind="ExternalInput")
    ident = nc.dram_tensor("ident", [128, 128], F32, kind="ExternalInput")
    if triu_mode == "host":
        out = nc.dram_tensor("out", [BPC, D, D], F32, kind="ExternalOutput")
    else:
        out = nc.dram_tensor("out", [BPC, TRI], F32, kind="ExternalOutput")

    # x[b] rows are (chunk, partition); one DMA per quarter-batch of 8 chunks
    xv = x.rearrange("b (h c p) d -> b h p c d", h=4, p=128)

    with TileContext(nc) as tc:
        with (
            tc.tile_pool(name="cst", bufs=1) as cst,
            tc.tile_pool(name="xin", bufs=10) as xinp,
            tc.tile_pool(name="sb", bufs=2) as sb,
            tc.tile_pool(name="ps", bufs=2, space="PSUM") as ps,
        ):
            ident_sb = cst.tile([128, 128], F32)
            nc.sync.dma_start(ident_sb, ident[:, :])
            ones2 = cst.tile([128, 2], F32)
            nc.vector.memset(ones2, 1.0)

            rnds = []
            for j in range(8):
                t = cst.tile([128, 258], F32R, name=f"rnd{j}")
                nc.vector.tensor_scalar_mul(t[:, 256:258], ones2, SCALE)
                rnds.append(t)

            # packed covariance halves for all 8 batches: [p, (b e)]
            covA = cst.tile([128, BPC * 256], F32)
            covB = cst.tile([128, BPC * 256], F32)

            pstate = {}

            def emit_chunks(b):
                ps0 = ps.tile([128, 258], F32, name=f"ps0_{b}", tag="ps0")
                ps1 = ps.tile([128, 258], F32, name=f"ps1_{b}", tag="ps1")
                quarters = []
                for h in range(4):
                    xt = xinp.tile([128, 8 * 256], F32, name=f"xt{b}_{h}",
                                   tag="xt")
                    nc.sync.dma_start(
                        xt.rearrange("p (c d) -> p c d", d=256), xv[b, h])
                    quarters.append(xt)
                for c in range(CHUNKS):
                    xt = quarters[c // 8]
                    sl = xt[:, (c % 8) * 256:(c % 8 + 1) * 256]
                    t = rnds[(b * CHUNKS + c) % len(rnds)]
                    nc.vector.tensor_scalar_mul(t[:, 0:256], sl, SCALE)
                    nc.tensor.matmul(ps0, t[:, 0:128], t, start=(c == 0),
                                     stop=False, skip_group_check=True)
                    nc.tensor.matmul(ps1, t[:, 128:256], t, start=(c == 0),
                                     stop=False, skip_group_check=True)
                pstate[b] = (ps0, ps1)

            def emit_epilogue(b):
                ps0, ps1 = pstate.pop(b)
                scol = sb.tile([128, 2], F32, name=f"scol{b}", tag="scol")
                nc.scalar.copy(scol[:, 0:1], ps0[:, 256:257])
                nc.scalar.copy(scol[:, 1:2], ps1[:, 256:257])
                pst = ps.tile([1, 256], F32, name=f"pst{b}", tag="pst")
                nc.tensor.transpose(pst[0:1, 0:128], scol[:, 0:1], ident_sb)
                nc.tensor.transpose(pst[0:1, 128:256], scol[:, 1:2], ident_sb)
                murow = sb.tile([1, 256], F32R, name=f"mur{b}", tag="mur")
                nmurow = sb.tile([1, 256], F32R, name=f"nmur{b}", tag="nmur")
                nc.scalar.copy(murow, pst[0:1, :])
                nc.scalar.mul(nmurow, pst[0:1, :], -1.0)
                nc.tensor.matmul(ps0[:, 0:256], nmurow[0:1, 0:128],
                                 murow[0:1, :], start=False, stop=True,
                                 skip_group_check=True)
                nc.tensor.matmul(ps1[:, 0:256], nmurow[0:1, 128:256],
                                 murow[0:1, :], start=False, stop=True,
                                 skip_group_check=True)
                nc.vector.tensor_copy(covA[:, b * 256:(b + 1) * 256],
                                      ps0[:, 0:256])
                nc.vector.tensor_copy(covB[:, b * 256:(b + 1) * 256],
                                      ps1[:, 0:256])
                if triu_mode == "host":
                    nc.sync.dma_start(out[b, 0:128, :],
                                      covA[:, b * 256:(b + 1) * 256])
                    nc.scalar.dma_start(out[b, 128:256, :],
                                        covB[:, b * 256:(b + 1) * 256])

            for b in range(BPC):
                emit_chunks(b)
                if b >= 1:
                    emit_epilogue(b - 1)
            emit_epilogue(BPC - 1)

            if triu_mode == "rowdma":
                covA3 = covA.rearrange("p (b e) -> p b e", e=256)
                covB3 = covB.rearrange("p (b e) -> p b e", e=256)
                engines = [nc.sync, nc.scalar]
                for d in range(D):
                    half = covA3 if d < 128 else covB3
                    p = d % 128
                    ln = D - d
                    off = d * D - (d * (d - 1)) // 2
                    src = half[p:p + 1, :, d:D]     # [1, 8, ln], 1 partition
                    dst = out[:, off:off + ln]      # [8, ln]
                    engines[d % 2].dma_start(dst, src)

    nc.finalize()
    return nc


def _get_nc(triu_mode):
    if triu_mode not in _cache:
        _cache[triu_mode] = _build(triu_mode)
    return _cache[triu_mode]


_TRIU_ROWS = None


def _host_gather(cov_full):
    # cov_full: [B, D, D] -> [B, TRI] row-major upper triangle
    global _TRIU_ROWS
    if _TRIU_ROWS is None:
        _TRIU_ROWS = np.triu_indices(D)
    iu, ju = _TRIU_ROWS
    return cov_full[:, iu, ju]


def kernel(**inputs):
    from concourse.bass_utils import run_bass_kernel_spmd

    x = np.asarray(inputs["inputs"], dtype=np.float32)
    assert x.shape == (B, N, D), x.shape
    ident = np.eye(128, dtype=np.float32)
    nc = _get_nc(TRIU_MODE)
    in_maps = [
        {"x": np.ascontiguousarray(x[c * BPC:(c + 1) * BPC]), "ident": ident}
        for c in range(NCORES)
    ]
    res = run_bass_kernel_spmd(nc, in_maps, core_ids=list(range(NCORES)))
    outs = [res.results[c]["out"] for c in range(NCORES)]
    full = np.concatenate(outs, axis=0)
    if TRIU_MODE == "host":
        return _host_gather(full)
    return full.reshape(B, TRI)


# revision 11
# speedup vs baseline: 2.2950x; 1.4659x over previous
"""Per-batch covariance + triu gather on 8 Trainium2 NeuronCores.

Problem: inputs [64, 4096, 256] f32 -> out [64, 32896] f32 where
out[b] = triu(cov(inputs[b])) in row-major order and
cov = (xc^T @ xc) / N with xc = x - mean(x, axis=0).

Strategy (data-parallel, 8 batches per core):
- y = x / 64 (exact scaling, 64 = sqrt(N)) is produced by the DVE pass that
  also rounds f32 -> f32r (required for single-pass-rate PE matmuls).
- An appended column of 1/64 makes the Gram accumulation produce the mean
  vector in the same matmuls: psum[d, 256] = sum_n y[n,d]/64 = mu[d], while
  psum[d, e] = sum_n y[n,d] y[n,e] = G[d,e]/N.
- mean correction: cov = G/N - mu mu^T is applied with one rank-1 matmul per
  128-row half accumulating into the same PSUM (lhsT = -mu, rhs = mu, K=1).
- triu extraction: either 256 row-tail DMAs (one per matrix row, covering all
  8 batches each) or a full-cov DMA with the gather done by the caller.
"""

import os
import numpy as np

B, N, D = 64, 4096, 256
NCORES = 8
BPC = B // NCORES          # batches per core
TRI = D * (D + 1) // 2     # 32896
CHUNKS = N // 128          # 32
SCALE = 1.0 / 64.0         # 1/sqrt(N)

TRIU_MODE = os.environ.get("COV_TRIU_MODE", "rowdma")  # "rowdma" | "host"

_cache = {}


def _build(triu_mode, reps=1, variant="base"):
    import concourse.bacc as bacc
    import concourse.mybir as mybir
    from concourse.tile import TileContext

    F32 = mybir.dt.float32
    F32R = mybir.dt.float32r

    nc = bacc.Bacc("TRN2", target_bir_lowering=False)
    x = nc.dram_tensor("x", [BPC, N, D], F32, kind="ExternalInput")
    ident = nc.dram_tensor("ident", [128, 128], F32, kind="ExternalInput")
    if triu_mode == "host":
        out = nc.dram_tensor("out", [BPC, D, D], F32, kind="ExternalOutput")
    else:
        out = nc.dram_tensor("out", [BPC, TRI], F32, kind="ExternalOutput")

    # x[b] rows are assigned to (quarter, partition, chunk) so each
    # partition's 8 rows are contiguous in DRAM (8KB descriptors). The
    # contraction over rows is order-invariant, so any bijective row
    # assignment is valid as long as lhsT/rhs read the same tile.
    xv = x.rearrange("b (h p c) d -> b h p c d", h=2, p=128)

    with TileContext(nc) as tc:
        with (
            tc.tile_pool(name="cst", bufs=1) as cst,
            tc.tile_pool(name="xin", bufs=5) as xinp,
            tc.tile_pool(name="sb", bufs=2) as sb,
            tc.tile_pool(name="ps", bufs=2, space="PSUM") as ps,
        ):
            ident_sb = cst.tile([128, 128], F32)
            nc.sync.dma_start(ident_sb, ident[:, :])
            ones2 = cst.tile([128, 2], F32)
            nc.vector.memset(ones2, 1.0)

            rnds = []
            for j in range(8):
                t = cst.tile([128, 258], F32R, name=f"rnd{j}")
                nc.vector.tensor_scalar_mul(t[:, 256:258], ones2, SCALE)
                rnds.append(t)

            # packed covariance halves for all 8 batches: [p, (b e)]
            covA = cst.tile([128, BPC * 256], F32)
            covB = cst.tile([128, BPC * 256], F32)

            pstate = {}

            def emit_chunks(key):
                rep, b = key
                ps0 = ps.tile([128, 258], F32, name=f"ps0_{rep}_{b}", tag="ps0")
                ps1 = ps.tile([128, 258], F32, name=f"ps1_{rep}_{b}", tag="ps1")
                quarters = []
                for h in range(2):
                    xt = xinp.tile([128, 16 * 256], F32, name=f"xt{rep}_{b}_{h}",
                                   tag="xt")
                    nc.sync.dma_start(
                        xt.rearrange("p (c d) -> p c d", d=256), xv[b, h])
                    quarters.append(xt)
                for c in range(CHUNKS):
                    xt = quarters[c // 16]
                    sl = xt[:, (c % 16) * 256:(c % 16 + 1) * 256]
                    t = rnds[(b * CHUNKS + c) % len(rnds)]
                    if variant == "dmapure":
                        continue
                    nc.vector.tensor_scalar_mul(t[:, 0:256], sl, SCALE)
                    if variant == "dmaonly":
                        continue
                    nc.tensor.matmul(ps0, t[:, 0:128], t, start=(c == 0),
                                     stop=False, skip_group_check=True)
                    nc.tensor.matmul(ps1, t[:, 128:256], t, start=(c == 0),
                                     stop=False, skip_group_check=True)
                pstate[key] = (ps0, ps1)

            def emit_epilogue(key):
                rep, b = key
                ps0, ps1 = pstate.pop(key)
                scol = sb.tile([128, 2], F32, name=f"scol{rep}_{b}", tag="scol")
                nc.scalar.copy(scol[:, 0:1], ps0[:, 256:257])
                nc.scalar.copy(scol[:, 1:2], ps1[:, 256:257])
                pst = ps.tile([1, 256], F32, name=f"pst{rep}_{b}", tag="pst")
                nc.tensor.transpose(pst[0:1, 0:128], scol[:, 0:1], ident_sb)
                nc.tensor.transpose(pst[0:1, 128:256], scol[:, 1:2], ident_sb)
                murow = sb.tile([1, 256], F32R, name=f"mur{rep}_{b}", tag="mur")
                nmurow = sb.tile([1, 256], F32R, name=f"nmur{rep}_{b}", tag="nmur")
                nc.scalar.copy(murow, pst[0:1, :])
                nc.scalar.mul(nmurow, pst[0:1, :], -1.0)
                nc.tensor.matmul(ps0[:, 0:256], nmurow[0:1, 0:128],
                                 murow[0:1, :], start=False, stop=True,
                                 skip_group_check=True)
                nc.tensor.matmul(ps1[:, 0:256], nmurow[0:1, 128:256],
                                 murow[0:1, :], start=False, stop=True,
                                 skip_group_check=True)
                nc.vector.tensor_copy(covA[:, b * 256:(b + 1) * 256],
                                      ps0[:, 0:256])
                nc.vector.tensor_copy(covB[:, b * 256:(b + 1) * 256],
                                      ps1[:, 0:256])
                if triu_mode == "host":
                    nc.sync.dma_start(out[b, 0:128, :],
                                      covA[:, b * 256:(b + 1) * 256])
                    nc.scalar.dma_start(out[b, 128:256, :],
                                        covB[:, b * 256:(b + 1) * 256])

            for rep in range(reps):
                for b in range(BPC):
                    emit_chunks((rep, b))
                    if variant in ("dmaonly", "dmapure"):
                        pstate.pop((rep, b))
                        continue
                    if b >= 1:
                        emit_epilogue((rep, b - 1))
                if variant not in ("dmaonly", "dmapure"):
                    emit_epilogue((rep, BPC - 1))

            if triu_mode == "rowdma":
                covA3 = covA.rearrange("p (b e) -> p b e", e=256)
                covB3 = covB.rearrange("p (b e) -> p b e", e=256)
                engines = [nc.sync, nc.scalar]
                for d in range(D):
                    half = covA3 if d < 128 else covB3
                    p = d % 128
                    ln = D - d
                    off = d * D - (d * (d - 1)) // 2
                    src = half[p:p + 1, :, d:D]     # [1, 8, ln], 1 partition
                    dst = out[:, off:off + ln]      # [8, ln]
                    engines[d % 2].dma_start(dst, src)

    nc.finalize()
    return nc


def _get_nc(triu_mode, reps=1, variant="base"):
    key = (triu_mode, reps, variant)
    if key not in _cache:
        _cache[key] = _build(triu_mode, reps, variant)
    return _cache[key]


_TRIU_ROWS = None


def _host_gather(cov_full):
    # cov_full: [B, D, D] -> [B, TRI] row-major upper triangle
    global _TRIU_ROWS
    if _TRIU_ROWS is None:
        _TRIU_ROWS = np.triu_indices(D)
    iu, ju = _TRIU_ROWS
    return cov_full[:, iu, ju]


def kernel(**inputs):
    from concourse.bass_utils import run_bass_kernel_spmd

    x = np.asarray(inputs["inputs"], dtype=np.float32)
    assert x.shape == (B, N, D), x.shape
    ident = np.eye(128, dtype=np.float32)
    nc = _get_nc(TRIU_MODE)
    in_maps = [
        {"x": np.ascontiguousarray(x[c * BPC:(c + 1) * BPC]), "ident": ident}
        for c in range(NCORES)
    ]
    res = run_bass_kernel_spmd(nc, in_maps, core_ids=list(range(NCORES)))
    outs = [res.results[c]["out"] for c in range(NCORES)]
    full = np.concatenate(outs, axis=0)
    if TRIU_MODE == "host":
        return _host_gather(full)
    return full.reshape(B, TRI)
